# revision 31
# baseline (speedup 1.0000x reference)
"""Trainium2 kernel for nn_BranchModel_9680856285960 (moe_routing).

Math: the reference scatters per-branch sparse weights into dense
(n_br, n_out, n_in) tensors, einsums against x, then takes a context-
gated masked sum over branches followed by relu.  Because the mask-
weighted branch sum commutes with the contraction over input features,
the whole model collapses to a 3-layer dense MLP

    out = relu(relu(x @ Weff1.T) @ Weff2.T) @ W3 + b3

where  Weff_l[o, i] = sum_{r,k} masks_l[ctx, r, o] * w_l[r, o, k]
                                * [idx_l[r, o, k] == i].

The effective-weight fold (a scatter-add over 5.6M index/value pairs) is
done once on the host; the device runs the dense pipeline.

Structural wins over a straight dense data-parallel mapping:
 * ~11% of hidden units have ALL branches masked (0.8^10), so their
   Weff rows are identically zero.  Those units are compacted away on
   the host (2000 -> ~1790, padded to a multiple of 128).
 * Matmuls run "flipped": the 128x128 weight tile is the stationary
   operand (LDWEIGHTS, FWL-pipelined) and the activation tile is the
   moving operand.  Layer outputs land feature-major in PSUM, so no
   on-chip transposes are needed anywhere.
 * Hybrid sharding, 4-way data-parallel x 2-way output-parallel: core
   (b, q) handles batch rows [256b, 256b+256) and the q-th half of the
   layer-2/3 output units.  W2/W3 are split in half per core, cutting
   the dominant weight stream from 9.5 MB to 6.6 MB per core; the two
   partial (10, 256) layer-3 outputs per batch shard are summed on the
   host during unsharding (layer 3 is linear, so the o-split commutes
   with the bias-free sum).  Layer 1 is computed in full per core
   (batch 256), which the wider N=256 moving operand absorbs.
 * A ~4us dummy-matmul warm-up defeats the HAM clock gate (PE
   otherwise runs the whole kernel at 1.2 GHz instead of 2.4).

No collectives.  Weight chunks stream on the two HWDGE rings in exact
consumption order; the PE trails the stream by at most one chunk.
"""

import os
import sys
import numpy as np

for _p in ("/opt/trn_rl_repo",):
    if os.path.isdir(_p) and _p not in sys.path:
        sys.path.append(_p)

from contextlib import ExitStack

from concourse import bass, mybir
import concourse.bacc as bacc
import concourse.tile as tile
from concourse.bass_utils import run_bass_kernel_spmd

F32 = mybir.dt.float32
F16 = mybir.dt.float16

BATCH, NIN, NH, NOUT = 1024, 784, 2000, 10
NCORES = 8
DP, OP = 4, 2                   # data-parallel x output-parallel grid
BS = BATCH // DP                # 256 batch rows per core
P = 128
K1F, K1R = NIN // P, NIN % P    # 6 full k-tiles of x + 16 ragged rows

# Exposed for the test harness: the BassKernelResults of the last run.
LAST_RESULT = None
_CACHE = {}


def _build_weff(w, idx, mask_row, n_in):
    """Fold masks + branch sum into a dense effective weight matrix.

    Weff[o, i] = sum_{r,k} mask_row[r, o] * w[r, o, k] * [idx[r, o, k] == i]
    """
    n_br, n_out, npb = w.shape
    acc = np.zeros(n_out * n_in, np.float64)
    base = (np.arange(n_out, dtype=np.int64) * n_in)[:, None]
    for r in range(n_br):
        flat = (base + idx[r].astype(np.int64)).ravel()
        vals = (w[r].astype(np.float64) * mask_row[r].astype(np.float64)[:, None]).ravel()
        acc += np.bincount(flat, weights=vals, minlength=n_out * n_in)
    return acc.reshape(n_out, n_in).astype(np.float32)


def _banks(nt, per):
    """Split nt o-tiles into PSUM banks of up to `per` tiles."""
    return [(q * per, min(per, nt - q * per)) for q in range((nt + per - 1) // per)]


def _chunks(nt):
    """Weight-stream chunking over i-tiles: pairs, then the last two solo
    (a small final chunk keeps the post-stream tail short)."""
    if nt <= 2:
        return [(t, 1) for t in range(nt)]
    out = []
    t = 0
    while t < nt - 2:
        c = min(2, nt - 2 - t)
        out.append((t, c))
        t += c
    out += [(nt - 2, 1), (nt - 1, 1)]
    return out


def _mlp_body(tc, nt1, ntq, xT, w1a, w1b, w2p, w3p, out):
    nc = tc.nc
    h1w = nt1 * P
    TB = BS // P                # batch tiles per core (2)
    # Matmuls run as N=128 batch-half passes: measured pair throughput is
    # 56 ns at N=128 (LDWEIGHTS fully hidden) vs 131 ns at N=256.  A PSUM
    # bank holds 4 [o-tile x batch-half] virtual tiles; v = to*TB + bh.
    b1 = _banks(nt1 * TB, 4)
    b2 = _banks(ntq * TB, 4)

    with ExitStack() as ctx:
        const = ctx.enter_context(tc.tile_pool(name="const", bufs=1))
        wp = ctx.enter_context(tc.tile_pool(name="wslab", bufs=1))
        act = ctx.enter_context(tc.tile_pool(name="act", bufs=1))
        pacc = ctx.enter_context(tc.tile_pool(name="pacc", bufs=1, space="PSUM"))

        # ---- PE warm-up: the HAM clock gate keeps the PE at 1.2 GHz until
        # it sees a ~3.4us fully-busy window.  The real matmuls are paced by
        # the weight stream and never present one, so without this the whole
        # kernel runs at half clock.  Burn ~4us of dummy matmuls (PE is idle
        # waiting on DMA anyway) to flip the gate before layer 1 starts.
        # The dummy bank shares its slot with the last L1 bank: dummies are
        # strictly earlier in PE program order, so the WAR dep is free.
        dum = const.tile([P, P], F16, tag="dum")
        nc.vector.memset(dum[:], 0.0)
        psd = pacc.tile([P, P], F32, tag=f"ps{len(b1) - 1}")
        for _ in range(34):
            nc.tensor.matmul(psd[:], lhsT=dum[:], rhs=dum[:],
                             start=True, stop=True)

        # ---- input
        xbig = const.tile([P, K1F + 1, BS], F16, tag="xbig")
        nc.sync.dma_start(out=xbig[:], in_=xT)

        w3t = const.tile([P, ntq, NOUT], F16, tag="w3")
        nc.gpsimd.dma_start(out=w3t[:], in_=w3p)

        # ---- weight stream, issued in exact consumption order, chunks
        # alternating between the two HWDGE rings (the final two w2 chunks
        # pinned to sync so the last chunk rides the wire alone).
        rings = [nc.sync, nc.scalar]
        ring_i = 1                      # k0 leads on scalar; x leads on sync

        w1s = []
        for k in range(K1F):
            slab = wp.tile([P, h1w], F16, name=f"w1s{k}", tag=f"w1s{k}")
            rings[ring_i].dma_start(out=slab[:], in_=w1a[:, k, :])
            ring_i ^= 1
            w1s.append(slab[:])
        w1bt = wp.tile([K1R, h1w], F16, tag="w1b")
        rings[ring_i].dma_start(out=w1bt[:], in_=w1b)
        ring_i ^= 1

        w2s = [None] * nt1
        for t0, cn in _chunks(nt1):
            slab = wp.tile([P, cn, ntq * P], F16, name=f"w2s{t0}", tag=f"w2s{t0}")
            ring = nc.sync if t0 >= nt1 - 2 else rings[ring_i]
            ring.dma_start(out=slab[:], in_=w2p[:, t0:t0 + cn, :])
            ring_i ^= 1
            for j in range(cn):
                w2s[t0 + j] = slab[:, j, :]

        # ---- Layer 1 (flipped): h1T[o, b] accumulated per o-tile in PSUM.
        # PSUM accumulation groups are per 2KB bank (zero region): only the
        # bank's first column-slice opens the group (start zeroes the whole
        # bank), siblings overwrite their still-pending-zero slice, and only
        # the bank's last slice at the final contraction step closes it.
        ps1 = [pacc.tile([P, n * P], F32, name=f"ps1_{q}", tag=f"ps{q}")
               for q, (_, n) in enumerate(b1)]
        h1q = [act.tile([P, n * P], F16, name=f"h1q{q}", tag=f"h1q{q}")
               for q, (_, n) in enumerate(b1)]
        for k in range(K1F):
            for to in range(nt1):
                for bh in range(TB):
                    v = to * TB + bh
                    q, j = v // 4, v % 4
                    nc.tensor.matmul(
                        ps1[q][:, j * P:(j + 1) * P],
                        lhsT=w1s[k][:, to * P:(to + 1) * P],
                        rhs=xbig[:P, k, bh * P:(bh + 1) * P],
                        start=(k == 0 and j == 0),
                        stop=False,
                    )
        # last contraction step bank-by-bank, each bank's relu issued right
        # behind its closing matmuls so layer 2 can start off bank 0 while
        # the PE finishes the remaining banks.
        for q, (v0, n) in enumerate(b1):
            for j in range(n):
                v = v0 + j
                nc.tensor.matmul(
                    ps1[q][:, j * P:(j + 1) * P],
                    lhsT=w1bt[:, (v // TB) * P:(v // TB + 1) * P],
                    rhs=xbig[:K1R, K1F, (v % TB) * P:(v % TB + 1) * P],
                    start=False,
                    stop=(j == n - 1),
                )
            # relu in two column halves so layer 2's first matmuls (which
            # need only the first vtiles) gate on the smaller first op
            h = (n * P) // 2
            nc.vector.tensor_scalar_max(h1q[q][:, :h], ps1[q][:, :h], 0.0)
            nc.vector.tensor_scalar_max(h1q[q][:, h:], ps1[q][:, h:], 0.0)

        # ---- Layer 2 (flipped): h2T[o, b] for this core's o-half.
        ps2 = [pacc.tile([P, n * P], F32, name=f"ps2_{q}", tag=f"ps{q}")
               for q, (_, n) in enumerate(b2)]
        h2q = [act.tile([P, n * P], F16, name=f"h2q{q}", tag=f"h2q{q}")
               for q, (_, n) in enumerate(b2)]

        def h1rhs(t, bh):
            v = t * TB + bh
            return h1q[v // 4][:, (v % 4) * P:(v % 4 + 1) * P]

        for t in range(nt1 - 1):
            for to in range(ntq):
                for bh in range(TB):
                    v = to * TB + bh
                    q, j = v // 4, v % 4
                    nc.tensor.matmul(
                        ps2[q][:, j * P:(j + 1) * P],
                        lhsT=w2s[t][:, to * P:(to + 1) * P],
                        rhs=h1rhs(t, bh),
                        start=(t == 0 and j == 0),
                        stop=False,
                    )
        # Final contraction step bank-by-bank; each bank's relu is split in
        # column halves across DVE and ACT so the two run concurrently, and
        # the layer-3 matmuls for that bank's vtiles follow immediately —
        # this chain (last matmuls -> relu -> layer 3 -> out) is the tail.
        # Layer-3 partials (outT_q = W3c[o-half].T @ h2T) accumulate into a
        # PSUM bank per batch half; the bias and the sum over the two
        # o-halves happen on the host during unshard.
        ps3h = [pacc.tile([NOUT, P], F32, name=f"ps3{bh}",
                          tag=f"ps{len(b2) + bh}") for bh in range(TB)]
        t = nt1 - 1
        nvt = ntq * TB
        for q, (v0, n) in enumerate(b2):
            for j in range(n):
                v = v0 + j
                nc.tensor.matmul(
                    ps2[q][:, j * P:(j + 1) * P],
                    lhsT=w2s[t][:, (v // TB) * P:(v // TB + 1) * P],
                    rhs=h1rhs(t, v % TB),
                    start=False,
                    stop=(j == n - 1),
                )
            h = (n * P) // 2
            nc.vector.tensor_scalar_max(h2q[q][:, :h], ps2[q][:, :h], 0.0)
            nc.scalar.activation(h2q[q][:, h:], ps2[q][:, h:],
                                 mybir.ActivationFunctionType.Relu)
            for j in range(n):
                v = v0 + j
                to, bh = v // TB, v % TB
                nc.tensor.matmul(
                    ps3h[bh][:],
                    lhsT=w3t[:, to, :],
                    rhs=h2q[q][:, j * P:(j + 1) * P],
                    start=(v == bh),
                    stop=(v == nvt - TB + bh),
                )
        # per-half PSUM drain and output DMA on separate engines/rings so
        # the two halves retire concurrently
        oh = [act.tile([NOUT, P], F32, name=f"o{bh}", tag=f"o{bh}")
              for bh in range(TB)]
        nc.vector.tensor_copy(oh[0][:], ps3h[0][:])
        nc.sync.dma_start(out=out[:, 0:P], in_=oh[0][:])
        nc.scalar.copy(oh[1][:], ps3h[1][:])
        nc.scalar.dma_start(out=out[:, P:BS], in_=oh[1][:])


def _get_program(nt1, ntq):
    key = (nt1, ntq)
    if key in _CACHE:
        return _CACHE[key]
    nc = bacc.Bacc("TRN2", target_bir_lowering=False, debug=False,
                   enable_asserts=False, enable_partition_id=False,
                   num_devices=NCORES)
    xT = nc.dram_tensor("xT", [P, K1F + 1, BS], F16,
                        kind="ExternalInput").ap()
    w1a = nc.dram_tensor("w1a", [P, K1F, nt1 * P], F16,
                         kind="ExternalInput").ap()
    w1b = nc.dram_tensor("w1b", [K1R, nt1 * P], F16,
                         kind="ExternalInput").ap()
    w2p = nc.dram_tensor("w2p", [P, nt1, ntq * P], F16,
                         kind="ExternalInput").ap()
    w3p = nc.dram_tensor("w3p", [P, ntq, NOUT], F16,
                         kind="ExternalInput").ap()
    out = nc.dram_tensor("out", [NOUT, BS], F32, kind="ExternalOutput").ap()
    with tile.TileContext(nc) as tc:
        _mlp_body(tc, nt1, ntq, xT, w1a, w1b, w2p, w3p, out)
    nc.compile()
    _CACHE[key] = nc
    return nc


def kernel(x, w1, idx1, w2, idx2, masks1, masks2, W3, b3, context):
    global LAST_RESULT
    x = np.ascontiguousarray(np.asarray(x, dtype=np.float32))
    ctxi = int(np.asarray(context))
    m1 = np.asarray(masks1)[ctxi]
    m2 = np.asarray(masks2)[ctxi]

    weff1 = _build_weff(np.asarray(w1), np.asarray(idx1), m1, NIN)
    weff2 = _build_weff(np.asarray(w2), np.asarray(idx2), m2, NH)

    # Compact away hidden units whose branches are all masked (zero rows).
    j1 = np.flatnonzero((m1 != 0).any(axis=0))
    j2 = np.flatnonzero((m2 != 0).any(axis=0))
    n1, n2 = len(j1), len(j2)
    nt1 = -(-n1 // P)
    ntq = -(-n2 // (P * OP))        # o-tiles per output-parallel half
    h1w, h2w = nt1 * P, ntq * P * OP

    w1cT = np.zeros((NIN, h1w), np.float32)
    w1cT[:, :n1] = weff1[j1].T
    w2cT = np.zeros((h1w, h2w), np.float32)
    w2cT[:n1, :n2] = weff2[np.ix_(j2, j1)].T
    w3c = np.zeros((h2w, NOUT), np.float32)
    w3c[:n2] = np.asarray(W3)[j2]

    w1a = np.ascontiguousarray(
        w1cT[:K1F * P].reshape(K1F, P, h1w).transpose(1, 0, 2)).astype(np.float16)
    w1b = np.ascontiguousarray(w1cT[K1F * P:NIN]).astype(np.float16)
    w2ps, w3ps = [], []
    for q in range(OP):
        cols = slice(q * ntq * P, (q + 1) * ntq * P)
        w2ps.append(np.ascontiguousarray(
            w2cT[:, cols].reshape(nt1, P, ntq * P).transpose(1, 0, 2)
        ).astype(np.float16))
        w3ps.append(np.ascontiguousarray(
            w3c[cols].reshape(ntq, P, NOUT).transpose(1, 0, 2)
        ).astype(np.float16))

    try:
        import antenv.axon_hooks  # noqa: F401
    except Exception:
        os.environ.setdefault("BASS_NEVER_TRACE", "1")

    nc = _get_program(nt1, ntq)
    in_maps = []
    for c in range(NCORES):
        b, q = c // OP, c % OP
        xs = x[b * BS:(b + 1) * BS].T.astype(np.float16)   # (784, 256)
        xT = np.zeros((P, K1F + 1, BS), np.float16)
        for k in range(K1F + 1):
            sz = P if k < K1F else K1R
            xT[:sz, k, :] = xs[k * P:k * P + sz, :]
        in_maps.append({"xT": xT, "w1a": w1a, "w1b": w1b,
                        "w2p": w2ps[q], "w3p": w3ps[q]})

    LAST_RESULT = run_bass_kernel_spmd(nc, in_maps, list(range(NCORES)))
    b3f = np.asarray(b3, dtype=np.float32)
    return np.concatenate(
        [(LAST_RESULT.results[b * OP]["out"] +
          LAST_RESULT.results[b * OP + 1]["out"]).T + b3f
         for b in range(DP)], axis=0)
